# revision 1
# baseline (speedup 1.0000x reference)
"""BEV cross-attention kernel for Trainium2, 8-core SPMD.

Shard: core c handles (batch b=c//4, head m=c%4). Full attention for one
(b, head): per-camera QK^T (Q=1024, K=6*1680), softmax over 10080 keys,
P@V, then partial output projection; AllReduce over the 4 cores of each
batch merges heads; final skip+LN+MLP+LN computed redundantly per group.

Layout strategy: feature-major ("S^T") attention — scores [k_chunk=128p,
q=1024f] so softmax exp runs on ScalarE with per-partition scale=rstd_k
(K LayerNorm) and bias=ln(rstd_v) (V LayerNorm folded through exp).
LayerNorm means are folded into projection weights host-side; the softmax
denominator rides the PV matmul as an all-ones column of V. No max
subtraction (logits are small by construction: |logit| < ~2).
"""
import numpy as np

import concourse.bass as bass
import concourse.bass_isa as bass_isa
import concourse.mybir as mybir
import concourse.tile as tile
from concourse.bass_utils import run_bass_kernel_spmd

F32 = mybir.dt.float32
F32R = mybir.dt.float32r

HEADS, DH, D = 4, 32, 128
B, NCAM = 2, 6
Q = 32 * 32            # 1024 BEV queries
KC = 28 * 60           # 1680 keys per camera
NKCH = (KC + 127) // 128   # 14 k-chunks per camera (last has 16 rows)
KFULL = KC // 128          # 13 full chunks
KTAIL = KC - KFULL * 128   # 16
N_CORES = 8
EPS = 1e-5
SCALE = DH ** -0.5

_cached = {}


# ---------------------------------------------------------------------------
# walrus compat: this container's walrus rejects instructions carrying more
# than one semaphore wait; move excess waits onto same-engine NoOps.
_COMPUTE_ENGINES = None
_nopctr = [0]


def _split_sync_waits(nc, limit=1):
    global _COMPUTE_ENGINES
    if _COMPUTE_ENGINES is None:
        _COMPUTE_ENGINES = {
            mybir.EngineType.PE, mybir.EngineType.Activation,
            mybir.EngineType.Pool, mybir.EngineType.DVE, mybir.EngineType.SP,
        }
    for f in nc.m.functions:
        for bb in f.blocks:
            out, changed = [], False
            for inst in bb.instructions:
                si = inst.sync_info
                if (si is not None and len(si.on_wait) > limit
                        and inst.engine in _COMPUTE_ENGINES):
                    waits = list(si.on_wait)
                    n_extra = len(waits) - limit
                    for i in range(0, n_extra, limit):
                        nop = mybir.InstNoOp(name=f"wait-split-{_nopctr[0]}")
                        _nopctr[0] += 1
                        nop.engine = inst.engine
                        nop.sync_info = mybir.SyncInfo(
                            on_wait=waits[i:min(i + limit, n_extra)], on_update=[])
                        out.append(nop)
                    si.on_wait = waits[n_extra:]
                    changed = True
                out.append(inst)
            if changed:
                bb.instructions = out


# ---------------------------------------------------------------------------
def _build_program(split=True, collective=True, n_dev=N_CORES):
    nc = bass.Bass("TRN2", target_bir_lowering=False, debug=False,
                   num_devices=n_dev)

    def din(name, shape, dt=F32R):
        return nc.dram_tensor(name, shape, dt, kind="ExternalInput").ap()

    xq = din("xq", [NCAM, D, Q])
    xk = din("xk", [NCAM, D, KC])
    xv = din("xv", [NCAM, D, KC])
    wq_ext = din("wq_ext", [D, 33])      # [s*Wq'' | ones/128]
    wk_ext = din("wk_ext", [D, 33])      # [Wk'' | ones/128]
    wv_ext = din("wv_ext", [D, 34])      # [Wv'' | zeros | ones/128]
    wbq = din("wbq", [32, 1], F32)       # s * Wq_m^T @ bq_ln
    wbv = din("wbv", [33, 1], F32)       # [Wv_m^T @ bv_ln | 0]
    wp = din("wp", [32, D])              # Wp head slice (lhsT)
    bp = din("bp", [D, 1], F32)
    skipb = din("skipb", [D, Q], F32)
    w1 = din("w1", [D, 256])
    b1 = din("b1", [2, D, 1], F32)
    w2 = din("w2", [D, 2, D])            # [ff128, half, dout]
    b2 = din("b2", [D, 1], F32)
    pre_g = din("pre_g", [D, 1], F32)
    pre_b = din("pre_b", [D, 1], F32)
    post_g = din("post_g", [D, 1], F32)
    post_b = din("post_b", [D, 1], F32)
    onesv = din("onesv", [1, D])

    out = nc.dram_tensor("out", [D, Q], F32, kind="ExternalOutput").ap()


    EXP = mybir.ActivationFunctionType.Exp
    LN_ = mybir.ActivationFunctionType.Ln
    SQRT = mybir.ActivationFunctionType.Sqrt
    GELU = mybir.ActivationFunctionType.Gelu

    with tile.TileContext(nc) as tc:
        with tc.tile_pool(name="consts", bufs=1) as consts, \
             tc.tile_pool(name="loads", bufs=2) as loads, \
             tc.tile_pool(name="sq", bufs=1) as sqp, \
             tc.tile_pool(name="rows", bufs=3) as rows, \
             tc.tile_pool(name="sml", bufs=4) as sml, \
             tc.tile_pool(name="keep", bufs=1) as keep, \
             tc.tile_pool(name="ee", bufs=3) as eep, \
             tc.tile_pool(name="fin", bufs=1) as finp, \
             tc.tile_pool(name="dramp", bufs=6, space="DRAM") as dramp:

            def row_split(row2d, t_f, width, nm, pool, dt=F32):
                """[1, N] SBUF row -> [128, width] token-major tile, via a
                DRAM bounce (partition<->free reshape is not one DMA)."""
                n_el = row2d.shape[1]
                dsc = dramp.tile([n_el], dt, name=nm + "_d", tag="dsc")
                nc.sync.dma_start(out=dsc, in_=row2d)
                t = pool.tile([128, width], dt, name=nm, tag=nm)
                full = n_el // 128
                nc.sync.dma_start(
                    out=t[:, 0:full],
                    in_=dsc[0:full * 128].rearrange("(c t) -> t c", t=128))
                tail = n_el - full * 128
                if tail:
                    nc.vector.memset(t[:, full:full + 1], 0.0)
                    nc.sync.dma_start(
                        out=t[0:tail, full:full + 1],
                        in_=dsc[full * 128:].rearrange("(c t) -> t c", t=tail))
                return t

            def tm_join(tm_tile, n_el, nm, dt):
                """[128, c] token-major tile -> [1, n_el] SBUF row via DRAM
                bounce."""
                dsc = dramp.tile([n_el], dt, name=nm + "_d", tag="dsc")
                nc.sync.dma_start(
                    out=dsc.rearrange("(c t) -> t c", t=128), in_=tm_tile)
                row = rows.tile([1, n_el], dt, name=nm, tag="row")
                nc.sync.dma_start(out=row, in_=dsc)
                return row

            # ---- constants ----
            wq_t = consts.tile([D, 33], F32R, name="wq_t")
            nc.sync.dma_start(out=wq_t, in_=wq_ext)
            wk_t = consts.tile([D, 33], F32R, name="wk_t")
            nc.sync.dma_start(out=wk_t, in_=wk_ext)
            wv_t = consts.tile([D, 34], F32R, name="wv_t")
            nc.sync.dma_start(out=wv_t, in_=wv_ext)
            wbq_t = consts.tile([32, 1], F32, name="wbq_t")
            nc.sync.dma_start(out=wbq_t, in_=wbq)
            wbv_t = consts.tile([33, 1], F32, name="wbv_t")
            nc.sync.dma_start(out=wbv_t, in_=wbv)
            wp_t = consts.tile([32, D], F32R, name="wp_t")
            nc.sync.dma_start(out=wp_t, in_=wp)
            onesbc = consts.tile([1, D], F32R, name="onesbc")
            nc.sync.dma_start(out=onesbc, in_=onesv)
            eps_t = consts.tile([D, 1], F32, name="eps_t")
            nc.vector.memset(eps_t, EPS)

            # ---- per-camera projections + stats ----
            qhT = keep.tile([33, NCAM, Q], F32R, name="qhT")
            khT = keep.tile([33, NCAM, KC], F32R, name="khT")
            vhE = keep.tile([D, NCAM, NKCH, 34], mybir.dt.bfloat16, name="vhE")
            rstdk = keep.tile([D, NCAM, NKCH], F32, name="rstdk")
            lnrv = keep.tile([D, NCAM, NKCH], F32, name="lnrv")

            ph1 = tc.tile_pool(name="proj", bufs=1, space="PSUM")
            projp = ph1.__enter__()
            ph1b = tc.tile_pool(name="stat", bufs=1, space="PSUM")
            statp = ph1b.__enter__()
            for n in range(NCAM):
                xq_t = loads.tile([D, Q], F32R, name="xq_t", tag="xq_t")
                nc.sync.dma_start(out=xq_t, in_=xq[n])
                xk_t = loads.tile([D, KC], F32R, name="xk_t", tag="xk_t")
                nc.sync.dma_start(out=xk_t, in_=xk[n])
                xv_t = loads.tile([D, KC], F32R, name="xv_t", tag="xv_t")
                nc.sync.dma_start(out=xv_t, in_=xv[n])

                x2q = sqp.tile([D, Q], F32R, name="x2q", tag="x2q")
                nc.vector.tensor_mul(out=x2q, in0=xq_t, in1=xq_t)
                x2k = sqp.tile([D, KC], F32R, name="x2k", tag="x2k")
                nc.vector.tensor_mul(out=x2k, in0=xk_t, in1=xk_t)
                x2v = sqp.tile([D, KC], F32R, name="x2v", tag="x2v")
                nc.vector.tensor_mul(out=x2v, in0=xv_t, in1=xv_t)

                # Q/K projections (feature-major), col 32 = mean
                qp_ps = projp.tile([33, Q], F32, name="qp_ps", tag="qp_ps")
                for h in range(2):
                    nc.tensor.matmul(qp_ps[:, h * 512:(h + 1) * 512],
                                     lhsT=wq_t, rhs=xq_t[:, h * 512:(h + 1) * 512],
                                     start=True, stop=True)
                for hh in range(2):
                    kp_ps = projp.tile([33, 2, 512], F32, name="kp_ps",
                                       tag="kp_ps")
                    for h2 in range(2):
                        h = hh * 2 + h2
                        nc.tensor.matmul(
                            kp_ps[:, h2, 0:420], lhsT=wk_t,
                            rhs=xk_t[:, h * 420:(h + 1) * 420],
                            start=True, stop=True)
                    nc.vector.tensor_copy(
                        out=khT[:, n, hh * 840:(hh + 1) * 840].rearrange(
                            "p (h c) -> p h c", h=2),
                        in_=kp_ps[:, :, 0:420])
                # V projection (token-major) col 33 = mean
                vp_ps = projp.tile([D, NKCH, 34], F32, name="vp_ps", tag="vp_ps")
                for c in range(NKCH):
                    cw = 128 if c < KFULL else KTAIL
                    nc.tensor.matmul(vp_ps[0:cw, c, :],
                                     lhsT=xv_t[:, c * 128:c * 128 + cw],
                                     rhs=wv_t, start=True, stop=True)

                # sum-of-squares rows via GpSimd cross-partition reduce (SBUF)
                ssq = rows.tile([1, Q], F32, name="ssq", tag="row")
                nc.gpsimd.tensor_reduce(out=ssq, in_=x2q,
                                        axis=mybir.AxisListType.C,
                                        op=mybir.AluOpType.add)
                ssk = rows.tile([1, KC], F32, name="ssk", tag="row")
                nc.gpsimd.tensor_reduce(out=ssk, in_=x2k,
                                        axis=mybir.AxisListType.C,
                                        op=mybir.AluOpType.add)
                ssv = rows.tile([1, KC], F32, name="ssv", tag="row")
                nc.gpsimd.tensor_reduce(out=ssv, in_=x2v,
                                        axis=mybir.AxisListType.C,
                                        op=mybir.AluOpType.add)

                # ---- evacuate projections to SBUF ----
                qraw = sqp.tile([33, Q], F32, name="qraw", tag="qraw")
                nc.vector.tensor_copy(out=qraw, in_=qp_ps)
                nc.vector.tensor_copy(out=vhE[:, n, 0:KFULL, :],
                                      in_=vp_ps[:, 0:KFULL, :])
                nc.vector.memset(vhE[:, n, KFULL, :], 0.0)
                nc.vector.tensor_copy(out=vhE[0:KTAIL, n, KFULL, :],
                                      in_=vp_ps[0:KTAIL, KFULL, :])
                # ones column for softmax denominator
                nc.vector.memset(vhE[:, n, :, 32], 1.0)

                # ---- token-major stats via reshape DMAs ----
                muq = row_split(qraw[32:33, :], 128, 8, "muq", sml)
                msqq = row_split(ssq, 128, 8, "msqq", sml)
                muk = row_split(khT[32:33, n, :].bitcast(F32), 128, NKCH,
                                "muk", sml)
                msqk = row_split(ssk, 128, NKCH, "msqk", sml)
                msqv = row_split(ssv, 128, NKCH, "msqv", sml)

                # ---- rstd computation (token-major, batched) ----
                def make_rstd(mu_ap, msq_ap, out_ap, width, log=False):
                    v = sml.tile([128, width], F32, name="v_rstd", tag="v_rstd")
                    nc.vector.tensor_mul(out=v, in0=mu_ap, in1=mu_ap)
                    v2 = sml.tile([128, width], F32, name="v2_rstd",
                                  tag="v2_rstd")
                    nc.vector.tensor_scalar_mul(out=v2, in0=msq_ap,
                                                scalar1=1.0 / 128.0)
                    nc.vector.tensor_sub(out=v, in0=v2, in1=v)
                    if log:
                        # ln(rstd) = -0.5*ln(var+eps)
                        nc.scalar.activation(out=v, in_=v, func=LN_,
                                             bias=eps_t, scale=1.0)
                        nc.vector.tensor_scalar_mul(out=out_ap, in0=v,
                                                    scalar1=-0.5)
                    else:
                        # rstd = exp(-0.5*ln(var+eps)): keeps ScalarE on the
                        # natural_log_exp table set (no sqrt-table reloads)
                        nc.scalar.activation(out=v, in_=v, func=LN_,
                                             bias=eps_t, scale=1.0)
                        if out_ap.dtype == F32R:
                            vexp = sml.tile([128, width], F32, name="vexp",
                                            tag="vexp")
                            nc.scalar.activation(out=vexp, in_=v, func=EXP,
                                                 bias=0.0, scale=-0.5)
                            nc.vector.tensor_copy(out=out_ap, in_=vexp)
                        else:
                            nc.scalar.activation(out=out_ap, in_=v, func=EXP,
                                                 bias=0.0, scale=-0.5)

                rstdq = sml.tile([128, 8], F32R, name="rstdq", tag="rstdq")
                make_rstd(muq, msqq, rstdq, 8)
                make_rstd(muk, msqk, rstdk[:, n, :], NKCH)
                # V mean lives token-major in vhE col 33
                muv = sml.tile([128, NKCH], F32, name="muv", tag="muv")
                nc.vector.tensor_copy(out=muv, in_=vhE[:, n, :, 33])
                make_rstd(muv, msqv, lnrv[:, n, :], NKCH, log=True)

                # ---- finalize qhT: rstd_q broadcast & apply ----
                rq_row = tm_join(rstdq, Q, "rq_row", F32R)
                rq_bc = statp.tile([32, Q], F32, name="rq_bc", tag="rq_bc")
                for h in range(2):
                    nc.tensor.matmul(rq_bc[:, h * 512:(h + 1) * 512],
                                     lhsT=onesbc[:, 0:32],
                                     rhs=rq_row[:, h * 512:(h + 1) * 512],
                                     start=True, stop=True)
                nc.vector.tensor_mul(out=qhT[0:32, n, :], in0=qraw[0:32, :],
                                     in1=rq_bc)
                nc.vector.tensor_scalar_add(out=qhT[0:32, n, :],
                                            in0=qhT[0:32, n, :], scalar1=wbq_t)

            ph1b.__exit__(None, None, None)
            ph1.__exit__(None, None, None)

            # ---- attention ----
            ph2 = tc.tile_pool(name="sc", bufs=2, space="PSUM")
            scp = ph2.__enter__()
            ph2b = tc.tile_pool(name="acc", bufs=1, space="PSUM")
            accp = ph2b.__enter__()
            avt = accp.tile([33, Q], F32, name="avt")  # accumulator, 2 banks
            first = True
            for n in range(NCAM):
                for c in range(NKCH):
                    cw = 128 if c < KFULL else KTAIL
                    sc_ps = scp.tile([128, Q], F32, name="sc_ps", tag="sc_ps")
                    # lhsT = khT chunk [32, cw]
                    kap = khT[0:32, n, :]
                    for h in range(2):
                        nc.tensor.matmul(
                            sc_ps[0:cw, h * 512:(h + 1) * 512],
                            lhsT=kap[:, c * 128:c * 128 + cw],
                            rhs=qhT[0:32, n, h * 512:(h + 1) * 512],
                            start=True, stop=True)
                    et = eep.tile([128, Q], mybir.dt.bfloat16, name="et", tag="et")
                    nc.scalar.activation(out=et[0:cw, :], in_=sc_ps[0:cw, :],
                                         func=EXP,
                                         bias=lnrv[0:cw, n, c:c + 1],
                                         scale=rstdk[0:cw, n, c:c + 1])
                    for h in range(2):
                        nc.tensor.matmul(
                            avt[:, h * 512:(h + 1) * 512],
                            lhsT=vhE[0:cw, n, c, 0:33],
                            rhs=et[0:cw, h * 512:(h + 1) * 512],
                            start=first, stop=(n == NCAM - 1 and c == NKCH - 1))
                    first = False

            # ---- normalize + output projection ----
            avt_sb = finp.tile([33, Q], F32, name="avt_sb", tag="f1")
            nc.vector.tensor_copy(out=avt_sb, in_=avt)
            ph2b.__exit__(None, None, None)
            ph2.__exit__(None, None, None)
            ph3 = tc.tile_pool(name="stat2", bufs=1, space="PSUM")
            st2p = ph3.__enter__()
            den = row_split(avt_sb[32:33, :], 128, 8, "den", sml)
            rden = sml.tile([128, 8], F32R, name="rden")
            with nc.allow_low_precision(reason="denominator rounding to f32r is intentional"):
                nc.vector.reciprocal(out=rden, in_=den)
            rd_row = tm_join(rden, Q, "rd_row", F32R)
            rd_bc = st2p.tile([32, Q], F32, name="rd_bc")
            for h in range(2):
                nc.tensor.matmul(rd_bc[:, h * 512:(h + 1) * 512],
                                 lhsT=onesbc[:, 0:32],
                                 rhs=rd_row[:, h * 512:(h + 1) * 512],
                                 start=True, stop=True)
            anorm = finp.tile([32, Q], F32R, name="anorm", tag="f3")
            nc.vector.tensor_mul(out=anorm, in0=avt_sb[0:32, :], in1=rd_bc)
            nc.vector.tensor_scalar_add(out=anorm, in0=anorm,
                                        scalar1=wbv_t[0:32, :])

            zp_ps = st2p.tile([D, Q], F32, name="zp_ps")
            for h in range(2):
                nc.tensor.matmul(zp_ps[:, h * 512:(h + 1) * 512], lhsT=wp_t,
                                 rhs=anorm[:, h * 512:(h + 1) * 512],
                                 start=True, stop=True)
            zp_sb = finp.tile([D, Q], F32, name="zp_sb", tag="f1")
            nc.vector.tensor_copy(out=zp_sb, in_=zp_ps)
            zpart = dramp.tile([D, Q], F32, name="zpart")
            zred = dramp.tile([D, Q], F32, name="zred")
            nc.sync.dma_start(out=zpart, in_=zp_sb)
            ph3.__exit__(None, None, None)

            if collective:
                nc.gpsimd.collective_compute(
                    "AllReduce", mybir.AluOpType.add,
                    replica_groups=[[0, 1, 2, 3], [4, 5, 6, 7]],
                    ins=[zpart.opt()], outs=[zred.opt()],
                )
            else:
                nc.sync.dma_start(out=zred, in_=zpart)
            ph4 = tc.tile_pool(name="fps", bufs=1, space="PSUM")
            fpsp = ph4.__enter__()

            # ---- final: skip + pre-LN + MLP + post-LN (redundant x4) ----
            w1_t = consts.tile([D, 256], F32R, name="w1_t")
            nc.sync.dma_start(out=w1_t, in_=w1)
            w2_t = consts.tile([D, 2, D], F32R, name="w2_t")
            nc.sync.dma_start(out=w2_t, in_=w2)
            b1_t = consts.tile([D, 2], F32, name="b1_t")
            nc.sync.dma_start(out=b1_t, in_=b1.rearrange("h d one -> d (h one)"))
            b2_t = consts.tile([D, 1], F32, name="b2_t")
            nc.sync.dma_start(out=b2_t, in_=b2)
            bp_t = consts.tile([D, 1], F32, name="bp_t")
            nc.sync.dma_start(out=bp_t, in_=bp)
            preg_t = consts.tile([D, 1], F32, name="preg_t")
            nc.sync.dma_start(out=preg_t, in_=pre_g)
            preb_t = consts.tile([D, 1], F32, name="preb_t")
            nc.sync.dma_start(out=preb_t, in_=pre_b)
            postg_t = consts.tile([D, 1], F32, name="postg_t")
            nc.sync.dma_start(out=postg_t, in_=post_g)
            postb_t = consts.tile([D, 1], F32, name="postb_t")
            nc.sync.dma_start(out=postb_t, in_=post_b)
            skip_t = consts.tile([D, Q], F32, name="skip_t")
            nc.sync.dma_start(out=skip_t, in_=skipb)

            zt = finp.tile([D, Q], F32R, name="zt")
            nc.sync.dma_start(out=zt.bitcast(F32), in_=zred)
            nc.vector.tensor_add(out=zt, in0=zt, in1=skip_t)
            nc.vector.tensor_scalar_add(out=zt, in0=zt, scalar1=bp_t)

            def feat_ln(src, gain, bias_, dst_dt, dst_name):
                """LayerNorm across partitions (d) of src [128, Q]."""
                s2 = finp.tile([D, Q], F32R, name=dst_name + "_s2",
                               tag="f2")
                nc.vector.tensor_mul(out=s2, in0=src, in1=src)
                srow = rows.tile([1, Q], F32, name=dst_name + "_srow",
                                 tag="row")
                nc.gpsimd.tensor_reduce(out=srow, in_=src,
                                        axis=mybir.AxisListType.C,
                                        op=mybir.AluOpType.add)
                s2row = rows.tile([1, Q], F32, name=dst_name + "_s2row",
                                  tag="row")
                nc.gpsimd.tensor_reduce(out=s2row, in_=s2,
                                        axis=mybir.AxisListType.C,
                                        op=mybir.AluOpType.add)
                mu_tm = row_split(srow, 128, 8, "ln_mu", sml)
                ms_tm = row_split(s2row, 128, 8, "ln_ms", sml)
                # mu = sum/128 ; var = sumsq/128 - mu^2
                mu2 = sml.tile([128, 8], F32R, name=dst_name + "_mu2",
                               tag="ln_mu2")
                nc.vector.tensor_scalar_mul(out=mu2, in0=mu_tm,
                                            scalar1=1.0 / 128.0)
                v = sml.tile([128, 8], F32, name=dst_name + "_v", tag="ln_v")
                nc.vector.tensor_mul(out=v, in0=mu2, in1=mu2)
                v2 = sml.tile([128, 8], F32, name=dst_name + "_v2",
                              tag="ln_v2")
                nc.vector.tensor_scalar_mul(out=v2, in0=ms_tm,
                                            scalar1=1.0 / 128.0)
                nc.vector.tensor_sub(out=v, in0=v2, in1=v)
                nc.scalar.activation(out=v, in_=v, func=LN_, bias=eps_t,
                                     scale=1.0)
                vexp = sml.tile([128, 8], F32, name=dst_name + "_ve",
                                tag="ln_ve")
                nc.scalar.activation(out=vexp, in_=v, func=EXP, bias=0.0,
                                     scale=-0.5)
                rs_tm = sml.tile([128, 8], F32R, name=dst_name + "_rs",
                                 tag="ln_rs")
                nc.vector.tensor_copy(out=rs_tm, in_=vexp)
                # rows back
                mu_row = tm_join(mu2, Q, dst_name + "_mur", F32R)
                rs_row = tm_join(rs_tm, Q, dst_name + "_rsr", F32R)
                mu_bc = fpsp.tile([D, Q], F32, name=dst_name + "_mubc",
                                  tag="ln_mubc")
                rs_bc = fpsp.tile([D, Q], F32, name=dst_name + "_rsbc",
                                  tag="ln_rsbc")
                for h in range(2):
                    nc.tensor.matmul(mu_bc[:, h * 512:(h + 1) * 512],
                                     lhsT=onesbc,
                                     rhs=mu_row[:, h * 512:(h + 1) * 512],
                                     start=True, stop=True)
                    nc.tensor.matmul(rs_bc[:, h * 512:(h + 1) * 512],
                                     lhsT=onesbc,
                                     rhs=rs_row[:, h * 512:(h + 1) * 512],
                                     start=True, stop=True)
                zc = finp.tile([D, Q], F32, name=dst_name + "_zc",
                               tag="f2")
                nc.vector.tensor_sub(out=zc, in0=src, in1=mu_bc)
                dst = finp.tile([D, Q], dst_dt, name=dst_name, tag="lndst")
                nc.vector.tensor_mul(out=dst, in0=zc, in1=rs_bc)
                nc.vector.tensor_scalar_mul(out=dst, in0=dst, scalar1=gain)
                nc.vector.tensor_scalar_add(out=dst, in0=dst, scalar1=bias_)
                return dst

            zhat = feat_ln(zt, preg_t, preb_t, F32R, "zhat")  # tag lndst

            # MLP: h^T = gelu(W1^T zhat + b1)
            gel = finp.tile([D, 2, Q], F32R, name="gel")
            for f in range(2):
                h_ps = fpsp.tile([D, Q], F32, name="h_ps", tag="h_ps")
                for h in range(2):
                    nc.tensor.matmul(h_ps[:, h * 512:(h + 1) * 512],
                                     lhsT=w1_t[:, f * 128:(f + 1) * 128],
                                     rhs=zhat[:, h * 512:(h + 1) * 512],
                                     start=True, stop=True)
                nc.scalar.activation(out=gel[:, f, :], in_=h_ps, func=GELU,
                                     bias=b1_t[:, f:f + 1], scale=1.0)
            o2_ps = fpsp.tile([D, Q], F32, name="o2_ps")
            for f in range(2):
                for h in range(2):
                    nc.tensor.matmul(o2_ps[:, h * 512:(h + 1) * 512],
                                     lhsT=w2_t[:, f, :],
                                     rhs=gel[:, f, h * 512:(h + 1) * 512],
                                     start=(f == 0), stop=(f == 1))
            res = finp.tile([D, Q], F32R, name="res")
            nc.vector.tensor_scalar_add(out=res, in0=o2_ps, scalar1=b2_t)
            nc.vector.tensor_add(out=res, in0=res, in1=zhat)

            final = feat_ln(res, postg_t, postb_t, F32, "final")
            nc.sync.dma_start(out=out, in_=final)
            ph4.__exit__(None, None, None)

    if split:
        _split_sync_waits(nc)
    return nc


# ---------------------------------------------------------------------------
def _prep_core_inputs(b, m, q, k, v, skip, q_ln_g, q_ln_b, Wq, bq, k_ln_g,
                      k_ln_b, Wk, bk, v_ln_g, v_ln_b, Wv, bv, Wp, bp,
                      pre_g, pre_b, W1, b1, W2, b2, post_g, post_b):
    f32 = np.float32
    sl = slice(m * DH, (m + 1) * DH)

    def fold(Wm, g):
        wg = (g[:, None] * Wm)
        return (wg - wg.sum(0, keepdims=True) / 128.0).astype(f32)

    wq_ext = np.zeros((D, 33), f32)
    wq_ext[:, 0:32] = SCALE * fold(Wq[:, sl], q_ln_g)
    wq_ext[:, 32] = 1.0 / 128.0
    wk_ext = np.zeros((D, 33), f32)
    wk_ext[:, 0:32] = fold(Wk[:, sl], k_ln_g)
    wk_ext[:, 32] = 1.0 / 128.0
    wv_ext = np.zeros((D, 34), f32)
    wv_ext[:, 0:32] = fold(Wv[:, sl], v_ln_g)
    wv_ext[:, 33] = 1.0 / 128.0

    wbq = (SCALE * (Wq[:, sl].T @ q_ln_b)).astype(f32).reshape(32, 1)
    wbv = np.zeros((33, 1), f32)
    wbv[0:32, 0] = Wv[:, sl].T @ v_ln_b

    return {
        "xq": np.ascontiguousarray(q[b].reshape(NCAM, D, Q), f32),
        "xk": np.ascontiguousarray(k[b].reshape(NCAM, D, KC), f32),
        "xv": np.ascontiguousarray(v[b].reshape(NCAM, D, KC), f32),
        "wq_ext": wq_ext, "wk_ext": wk_ext, "wv_ext": wv_ext,
        "wbq": wbq, "wbv": wbv,
        "wp": np.ascontiguousarray(Wp[sl, :], f32),
        "bp": bp.astype(f32).reshape(D, 1),
        "skipb": np.ascontiguousarray(skip[b].reshape(D, Q), f32),
        "w1": W1.astype(f32),
        "b1": b1.astype(f32).reshape(2, D, 1),
        "w2": np.ascontiguousarray(
            W2.reshape(2, D, D).transpose(1, 0, 2), f32),
        "b2": b2.astype(f32).reshape(D, 1),
        "pre_g": pre_g.astype(f32).reshape(D, 1),
        "pre_b": pre_b.astype(f32).reshape(D, 1),
        "post_g": post_g.astype(f32).reshape(D, 1),
        "post_b": post_b.astype(f32).reshape(D, 1),
        "onesv": np.ones((1, D), f32),
    }


def kernel(**inputs):
    if "nc" not in _cached:
        _cached["nc"] = _build_program()
    nc = _cached["nc"]
    args = {kk: np.asarray(vv) for kk, vv in inputs.items()}
    in_maps = [_prep_core_inputs(c // 4, c % 4, **args) for c in range(N_CORES)]
    res = run_bass_kernel_spmd(nc, in_maps, core_ids=list(range(N_CORES)))
    out = np.stack([res.results[0]["out"], res.results[4]["out"]])
    return out.reshape(B, D, 32, 32)



# revision 10
# speedup vs baseline: 1.4381x; 1.4381x over previous
"""BEV cross-attention kernel for Trainium2, 8-core SPMD.

Shard: core c handles (batch b=c//4, head m=c%4). Full attention for one
(b, head): per-camera QK^T (Q=1024, K=6*1680), softmax over 10080 keys,
P@V, partial output projection; AllReduce over the 4 cores of each batch
merges heads; final skip+LN+MLP+LN computed redundantly per group.

v2 layout strategy (vs v1): all LayerNorm statistics are produced
token-major directly by tiny PE matmuls (lhsT=x chunk, rhs=ones col) --
no gpsimd cross-partition reduces and no DRAM reshape bounces. Q is
projected token-major, scaled by rstd_q per-partition, and transposed
back with PE is_transpose ops. The Q-side projection bias rides the exp
bias via u = Wk_ext @ wbq (logit bias per key), combined with ln(rstd_v)
(V LayerNorm folded through exp). Softmax denominator rides the PV
matmul as an all-ones column of V. Per-camera stages are pipelined
(load n+1 / project n / attend n-1) and the QK->exp->PV chain is
software-pipelined so the in-order PE queue never head-of-line blocks
on the Activation engine. All constants arrive in one packed DMA.
"""
import numpy as np

import concourse.bass as bass
import concourse.bass_isa as bass_isa
import concourse.mybir as mybir
import concourse.tile as tile
from concourse.bass_utils import run_bass_kernel_spmd

F32 = mybir.dt.float32
F32R = mybir.dt.float32r
BF16 = mybir.dt.bfloat16

HEADS, DH, D = 4, 32, 128
B, NCAM = 2, 6
Q = 32 * 32            # 1024 BEV queries
NQCH = Q // 128        # 8 q-chunks
KC = 28 * 60           # 1680 keys per camera
NKCH = (KC + 127) // 128   # 14 k-chunks per camera (last has 16 rows)
KFULL = KC // 128          # 13 full chunks
KTAIL = KC - KFULL * 128   # 16
N_CORES = 8
EPS = 1e-5
SCALE = DH ** -0.5

# packed weight tensor column map
CW_Q = 0           # 33 cols: s*fold(Wq g) | 1/128
CW_K = 33          # 32 cols: fold(Wk g)
CW_V = 65          # 34 cols: fold(Wv g) | 0 | 1/128
CW_KM = 99         # 2 cols: 1/128 | u  (k-mean, wbq.kh bias)
CW_ONE = 101       # 1 col: ones
CW_W1 = 104        # 256 cols: pre_g-folded W1
CW_W2 = 360        # 256 cols: W2 (two 128-row chunks)
CW_BP = 616        # f32 cols (bitcast): bp
CW_BC2 = 617       # pre_b + b2
CW_POG = 618       # post_g
CW_POB = 619       # post_b
CW_PRG = 620       # pre_g
CW_B1A = 621       # b1' half 0
CW_B1B = 622       # b1' half 1
CW_WBV = 623       # wbv (rows 0:32)
W_TOT = 624

_cached = {}


# ---------------------------------------------------------------------------
# walrus compat: this container's walrus rejects instructions carrying more
# than one semaphore wait; move excess waits onto same-engine NoOps.
_COMPUTE_ENGINES = None
_nopctr = [0]


def _split_sync_waits(nc, limit=1):
    global _COMPUTE_ENGINES
    if _COMPUTE_ENGINES is None:
        _COMPUTE_ENGINES = {
            mybir.EngineType.PE, mybir.EngineType.Activation,
            mybir.EngineType.Pool, mybir.EngineType.DVE, mybir.EngineType.SP,
        }
    for f in nc.m.functions:
        for bb in f.blocks:
            out, changed = [], False
            for inst in bb.instructions:
                si = inst.sync_info
                if (si is not None and len(si.on_wait) > limit
                        and inst.engine in _COMPUTE_ENGINES):
                    waits = list(si.on_wait)
                    n_extra = len(waits) - limit
                    for i in range(0, n_extra, limit):
                        nop = mybir.InstNoOp(name=f"wait-split-{_nopctr[0]}")
                        _nopctr[0] += 1
                        nop.engine = inst.engine
                        nop.sync_info = mybir.SyncInfo(
                            on_wait=waits[i:min(i + limit, n_extra)], on_update=[])
                        out.append(nop)
                    si.on_wait = waits[n_extra:]
                    changed = True
                out.append(inst)
            if changed:
                bb.instructions = out
    return nc


# ---------------------------------------------------------------------------
def _build_program(split=True, collective=True, n_dev=N_CORES):
    nc = bass.Bass("TRN2", target_bir_lowering=False, debug=False,
                   num_devices=n_dev)

    def din(name, shape, dt=F32R):
        return nc.dram_tensor(name, shape, dt, kind="ExternalInput").ap()

    xq = din("xq", [NCAM, D, Q])
    xk = din("xk", [NCAM, D, KC])
    xv = din("xv", [NCAM, D, KC])
    wbig = din("wbig", [D, W_TOT])
    wp = din("wp", [32, D])              # Wp head slice (lhsT)
    skipb = din("skipb", [D, Q], F32)
    eye = din("eye", [D, D])

    out = nc.dram_tensor("out", [D, Q], F32, kind="ExternalOutput").ap()

    EXP = mybir.ActivationFunctionType.Exp
    LN_ = mybir.ActivationFunctionType.Ln
    SQRT = mybir.ActivationFunctionType.Sqrt
    GELU = mybir.ActivationFunctionType.Gelu
    ADD = mybir.AluOpType.add
    SUB = mybir.AluOpType.subtract
    MUL = mybir.AluOpType.mult

    with tile.TileContext(nc) as tc:
        with tc.tile_pool(name="consts", bufs=1) as consts, \
             tc.tile_pool(name="loads", bufs=2) as loads, \
             tc.tile_pool(name="sq", bufs=2) as sqp, \
             tc.tile_pool(name="sml", bufs=2) as sml, \
             tc.tile_pool(name="keep", bufs=1) as keep, \
             tc.tile_pool(name="qts", bufs=2) as qts, \
             tc.tile_pool(name="ee", bufs=3) as eep, \
             tc.tile_pool(name="fin", bufs=1) as finp, \
             tc.tile_pool(name="dramp", bufs=2, space="DRAM") as dramp, \
             tc.tile_pool(name="pps", bufs=2, space="PSUM") as pps, \
             tc.tile_pool(name="pacc", bufs=1, space="PSUM") as pacc, \
             tc.tile_pool(name="pvp", bufs=1, space="PSUM") as pvp, \
             tc.tile_pool(name="pqp", bufs=1, space="PSUM") as pqp:

            # ---- constants (one big DMA + tiny ones) ----
            wb = consts.tile([D, W_TOT], F32R, name="wb")
            nc.sync.dma_start(out=wb, in_=wbig)
            wp_t = consts.tile([32, D], F32R, name="wp_t")
            nc.sync.dma_start(out=wp_t, in_=wp)
            skip_t = consts.tile([D, Q], F32, name="skip_t")
            nc.sync.dma_start(out=skip_t, in_=skipb)
            eye_t = consts.tile([D, D], F32R, name="eye_t")
            nc.sync.dma_start(out=eye_t, in_=eye)
            onesbc = consts.tile([1, D], F32R, name="onesbc")
            nc.vector.memset(onesbc, 1.0)

            def bcol(c, p=D):
                return wb[0:p, c:c + 1].bitcast(F32)

            # ---- persistent activations ----
            qhT = keep.tile([32, NCAM, Q], BF16, name="qhT")
            khT = keep.tile([32, NCAM, KC], BF16, name="khT")
            vhE = keep.tile([D, NCAM, NKCH, 33], BF16, name="vhE")
            rstdk = keep.tile([D, NCAM, NKCH], F32, name="rstdk")
            biask = keep.tile([D, NCAM, NKCH], F32, name="biask")

            # qp psum layout: [0:264) = 8 q-chunks x 33 (proj+mean),
            # [264:292) k mean/wbqk pairs, [292:306) k sumsq,
            # [306:320) v sumsq, [320:328) q sumsq
            QP_KM = 264
            QP_KS = 292
            QP_VS = 306
            QP_QS = 320

            loaded = {}

            def stage_load(n):
                xq_t = loads.tile([D, Q], F32R, name="xq_t", tag="xq_t")
                nc.sync.dma_start(out=xq_t, in_=xq[n])
                xk_t = loads.tile([D, KC], F32R, name="xk_t", tag="xk_t")
                nc.sync.dma_start(out=xk_t, in_=xk[n])
                xv_t = loads.tile([D, KC], F32R, name="xv_t", tag="xv_t")
                nc.sync.dma_start(out=xv_t, in_=xv[n])
                # squares: q+k on gpsimd, v on DVE (engine balance)
                x2q = sqp.tile([D, Q], F32R, name="x2q", tag="x2q")
                nc.gpsimd.scalar_tensor_tensor(
                    out=x2q, in0=xq_t, scalar=1.0, in1=xq_t, op0=MUL, op1=MUL)
                x2k = sqp.tile([D, KC], F32R, name="x2k", tag="x2k")
                nc.gpsimd.scalar_tensor_tensor(
                    out=x2k, in0=xk_t, scalar=1.0, in1=xk_t, op0=MUL, op1=MUL)
                x2v = sqp.tile([D, KC], F32R, name="x2v", tag="x2v")
                nc.vector.tensor_mul(out=x2v, in0=xv_t, in1=xv_t)
                loaded[n] = (xq_t, xk_t, xv_t, x2q, x2k, x2v)

            def stage_proj(n):
                xq_t, xk_t, xv_t, x2q, x2k, x2v = loaded.pop(n)
                qp = pqp.tile([D, 328], F32, name="qp", tag="qp")
                # Q proj token-major + q mean (col 32 of each 33-group)
                for c in range(NQCH):
                    nc.tensor.matmul(
                        qp[:, c * 33:(c + 1) * 33],
                        lhsT=xq_t[:, c * 128:(c + 1) * 128],
                        rhs=wb[:, CW_Q:CW_Q + 33], start=True, stop=True)
                    nc.tensor.matmul(
                        qp[:, QP_QS + c:QP_QS + c + 1],
                        lhsT=x2q[:, c * 128:(c + 1) * 128],
                        rhs=wb[:, CW_ONE:CW_ONE + 1], start=True, stop=True)
                # K/V stats
                for c in range(NKCH):
                    cw = 128 if c < KFULL else KTAIL
                    sl = slice(c * 128, c * 128 + cw)
                    nc.tensor.matmul(
                        qp[0:cw, QP_KM + 2 * c:QP_KM + 2 * c + 2],
                        lhsT=xk_t[:, sl], rhs=wb[:, CW_KM:CW_KM + 2],
                        start=True, stop=True)
                    nc.tensor.matmul(
                        qp[0:cw, QP_KS + c:QP_KS + c + 1],
                        lhsT=x2k[:, sl], rhs=wb[:, CW_ONE:CW_ONE + 1],
                        start=True, stop=True)
                    nc.tensor.matmul(
                        qp[0:cw, QP_VS + c:QP_VS + c + 1],
                        lhsT=x2v[:, sl], rhs=wb[:, CW_ONE:CW_ONE + 1],
                        start=True, stop=True)
                # K projection (feature-major) in two [32, 2, 512]-shaped
                # psum slots of the sc ring
                for hh in range(2):
                    kp = pps.tile([32, 2, 512], F32, name="kp", tag="sc")
                    for h2 in range(2):
                        h = hh * 2 + h2
                        nc.tensor.matmul(
                            kp[:, h2, 0:420], lhsT=wb[:, CW_K:CW_K + 32],
                            rhs=xk_t[:, h * 420:(h + 1) * 420],
                            start=True, stop=True)
                    nc.vector.tensor_copy(
                        out=khT[:, n, hh * 840:(hh + 1) * 840].rearrange(
                            "p (h c) -> p h c", h=2),
                        in_=kp[:, :, 0:420])
                # V projection (token-major), col 32 = 0 (-> ones), 33 = mean
                vp = pvp.tile([D, NKCH, 34], F32, name="vp", tag="vp")
                for c in range(NKCH):
                    cw = 128 if c < KFULL else KTAIL
                    nc.tensor.matmul(vp[0:cw, c, :],
                                     lhsT=xv_t[:, c * 128:c * 128 + cw],
                                     rhs=wb[:, CW_V:CW_V + 34],
                                     start=True, stop=True)
                nc.vector.tensor_copy(out=vhE[:, n, :, 0:32],
                                      in_=vp[:, :, 0:32])
                nc.vector.memset(vhE[:, n, :, 32], 1.0)

                # ---- rstd computations (token-major, small) ----
                def make_var(mu_ap, msq_ap, width, nm):
                    mu2 = sml.tile([128, width], F32, name=nm + "_mu2",
                                   tag=nm + "_mu2")
                    nc.vector.tensor_mul(out=mu2, in0=mu_ap, in1=mu_ap)
                    v = sml.tile([128, width], F32, name=nm + "_v",
                                 tag=nm + "_v")
                    nc.vector.scalar_tensor_tensor(
                        out=v, in0=msq_ap, scalar=1.0 / 128.0, in1=mu2,
                        op0=MUL, op1=SUB)
                    nc.vector.tensor_scalar_add(out=v, in0=v, scalar1=EPS)
                    return v

                # K: rstd_k = sqrt(1/var)
                kvar = make_var(
                    qp[:, QP_KM:QP_KM + 28].rearrange("p (c two) -> p two c",
                                                      two=2)[:, 0, :],
                    qp[:, QP_KS:QP_KS + NKCH], NKCH, "k")
                krec = sml.tile([128, NKCH], F32, name="krec", tag="krec")
                nc.vector.reciprocal(out=krec, in_=kvar)
                nc.scalar.activation(out=rstdk[:, n, :], in_=krec, func=SQRT)
                # V: bias = wbqk - 0.5*ln(var_v) (= wbqk + ln rstd_v)
                vvar = make_var(vp[:, :, 33], qp[:, QP_VS:QP_VS + NKCH],
                                NKCH, "v")
                vln = sml.tile([128, NKCH], F32, name="vln", tag="vln")
                nc.scalar.activation(out=vln, in_=vvar, func=LN_)
                nc.vector.scalar_tensor_tensor(
                    out=biask[:, n, :], in0=vln, scalar=-0.5,
                    in1=qp[:, QP_KM:QP_KM + 28].rearrange(
                        "p (c two) -> p two c", two=2)[:, 1, :],
                    op0=MUL, op1=ADD)
                # Q: rstd_q, then scale token-major projections
                qvar = make_var(
                    qp[:, 0:264].rearrange("p (c w) -> p w c", w=33)[:, 32, :],
                    qp[:, QP_QS:QP_QS + NQCH], NQCH, "q")
                qrec = sml.tile([128, NQCH], F32, name="qrec", tag="qrec")
                nc.vector.reciprocal(out=qrec, in_=qvar)
                rstdq = sml.tile([128, NQCH], F32, name="rstdq", tag="rstdq")
                nc.scalar.activation(out=rstdq, in_=qrec, func=SQRT)
                qpTs = qts.tile([D, NQCH, 32], F32R, name="qpTs", tag="qpTs")
                for c in range(NQCH):
                    nc.vector.tensor_scalar_mul(
                        out=qpTs[:, c, :], in0=qp[:, c * 33:c * 33 + 32],
                        scalar1=rstdq[:, c:c + 1])
                # transpose back to feature-major [32, Q]
                qT = pps.tile([32, Q], F32R, name="qT", tag="sc")
                for c in range(NQCH):
                    nc.tensor.transpose(out=qT[:, c * 128:(c + 1) * 128],
                                        in_=qpTs[:, c, :], identity=eye_t)
                nc.vector.tensor_copy(out=qhT[:, n, :], in_=qT)

            # ---- attention chunk list, software pipelined ----
            avt = pacc.tile([33, Q], F32, name="avt")
            chunks = [(n, c) for n in range(NCAM) for c in range(NKCH)]
            pend = []  # (et, n, c, first)

            def emit_qk(i):
                n, c = chunks[i]
                cw = 128 if c < KFULL else KTAIL
                sc = pps.tile([128, Q], F32, name="sc", tag="sc")
                for h in range(2):
                    nc.tensor.matmul(
                        sc[0:cw, h * 512:(h + 1) * 512],
                        lhsT=khT[:, n, c * 128:c * 128 + cw],
                        rhs=qhT[:, n, h * 512:(h + 1) * 512],
                        start=True, stop=True)
                et = eep.tile([128, Q], BF16, name="et", tag="et")
                nc.scalar.activation(out=et[0:cw, :], in_=sc[0:cw, :],
                                     func=EXP,
                                     bias=biask[0:cw, n, c:c + 1],
                                     scale=rstdk[0:cw, n, c:c + 1])
                pend.append((et, n, c, i == 0))

            def emit_pv(last=False):
                et, n, c, first = pend.pop(0)
                cw = 128 if c < KFULL else KTAIL
                for h in range(2):
                    nc.tensor.matmul(
                        avt[:, h * 512:(h + 1) * 512],
                        lhsT=vhE[0:cw, n, c, 0:33],
                        rhs=et[0:cw, h * 512:(h + 1) * 512],
                        start=first, stop=last)

            # ---- stage-pipelined main loop ----
            stage_load(0)
            stage_proj(0)
            stage_load(1)
            ci = 0
            for n in range(NCAM):
                for _ in range(NKCH):
                    emit_qk(ci)
                    ci += 1
                    if len(pend) > 1:
                        emit_pv(last=False)
                if n + 1 < NCAM:
                    stage_proj(n + 1)
                if n + 2 < NCAM:
                    stage_load(n + 2)
            while pend:
                emit_pv(last=len(pend) == 1)

            # ---- normalize + output projection ----
            avt_sb = finp.tile([33, Q], F32, name="avt_sb")
            nc.vector.tensor_copy(out=avt_sb, in_=avt)
            rden = finp.tile([1, Q], F32, name="rden", tag="lnrs")
            nc.vector.reciprocal(out=rden, in_=avt_sb[32:33, :])
            rd_bc = pps.tile([32, Q], F32, name="rd_bc", tag="sc")
            for h in range(2):
                nc.tensor.matmul(rd_bc[:, h * 512:(h + 1) * 512],
                                 lhsT=onesbc[:, 0:32],
                                 rhs=rden[:, h * 512:(h + 1) * 512].bitcast(F32R),
                                 start=True, stop=True)
            anorm = finp.tile([32, Q], F32R, name="anorm")
            nc.vector.tensor_mul(out=anorm, in0=avt_sb[0:32, :], in1=rd_bc)
            nc.vector.tensor_scalar_add(out=anorm, in0=anorm,
                                        scalar1=bcol(CW_WBV, 32))
            zp = pps.tile([D, Q], F32, name="zp", tag="sc")
            for h in range(2):
                nc.tensor.matmul(zp[:, h * 512:(h + 1) * 512], lhsT=wp_t,
                                 rhs=anorm[:, h * 512:(h + 1) * 512],
                                 start=True, stop=True)
            zp_sb = finp.tile([D, Q], F32, name="zp_sb")
            nc.vector.tensor_copy(out=zp_sb, in_=zp)
            zpart = dramp.tile([D, Q], F32, name="zpart")
            zred = dramp.tile([D, Q], F32, name="zred")
            nc.sync.dma_start(out=zpart, in_=zp_sb)
            if collective:
                nc.gpsimd.collective_compute(
                    "AllReduce", mybir.AluOpType.add,
                    replica_groups=[[0, 1, 2, 3], [4, 5, 6, 7]],
                    ins=[zpart.opt()], outs=[zred.opt()],
                )
            else:
                nc.sync.dma_start(out=zred, in_=zpart)

            # ---- final: skip + pre-LN + MLP + post-LN (redundant x4) ----
            zt = finp.tile([D, Q], F32, name="zt")
            nc.sync.dma_start(out=zt, in_=zred)
            zsum = finp.tile([D, Q], F32R, name="zsum")
            nc.vector.scalar_tensor_tensor(
                out=zsum, in0=zt, scalar=bcol(CW_BP), in1=skip_t,
                op0=ADD, op1=ADD)

            def feat_ln_raw(src, nm):
                """(src - mu) * rstd over partitions (d); row-stat form."""
                s2 = finp.tile([D, Q], F32R, name=nm + "_s2", tag="lns2")
                nc.vector.tensor_mul(out=s2, in0=src, in1=src)
                srow = pps.tile([33, Q], F32, name=nm + "_srow", tag="sc")
                for h in range(2):
                    hs = slice(h * 512, (h + 1) * 512)
                    nc.tensor.matmul(srow[0:1, hs],
                                     lhsT=wb[:, CW_ONE:CW_ONE + 1],
                                     rhs=src[:, hs], start=True, stop=True)
                    nc.tensor.matmul(srow[32:33, hs],
                                     lhsT=wb[:, CW_ONE:CW_ONE + 1],
                                     rhs=s2[:, hs], start=True, stop=True)
                rows = finp.tile([4, Q], F32, name=nm + "_rows",
                                 tag="rows")
                mu = rows[0:1, :]
                nc.vector.tensor_scalar_mul(out=mu, in0=srow[0:1, :],
                                            scalar1=1.0 / 128.0)
                mu2 = rows[1:2, :]
                nc.vector.tensor_mul(out=mu2, in0=mu, in1=mu)
                var = rows[2:3, :]
                nc.vector.scalar_tensor_tensor(
                    out=var, in0=srow[32:33, :], scalar=1.0 / 128.0, in1=mu2,
                    op0=MUL, op1=SUB)
                nc.vector.tensor_scalar_add(out=var, in0=var, scalar1=EPS)
                rec = rows[3:4, :]
                nc.vector.reciprocal(out=rec, in_=var)
                rs = finp.tile([1, Q], F32, name=nm + "_rs", tag="lnrs")
                nc.scalar.activation(out=rs, in_=rec, func=SQRT)
                mu_bc = pps.tile([D, Q], F32, name=nm + "_mubc", tag="sc")
                rs_bc = pps.tile([D, Q], F32, name=nm + "_rsbc", tag="sc")
                for h in range(2):
                    hs = slice(h * 512, (h + 1) * 512)
                    nc.tensor.matmul(mu_bc[:, hs], lhsT=onesbc,
                                     rhs=mu[:, hs].bitcast(F32R),
                                     start=True, stop=True)
                    nc.tensor.matmul(rs_bc[:, hs], lhsT=onesbc,
                                     rhs=rs[:, hs].bitcast(F32R),
                                     start=True, stop=True)
                zc = finp.tile([D, Q], F32, name=nm + "_zc", tag="lnzc")
                nc.vector.tensor_sub(out=zc, in0=src, in1=mu_bc)
                dst = finp.tile([D, Q], F32R, name=nm, tag="lndst")
                nc.vector.tensor_mul(out=dst, in0=zc, in1=rs_bc)
                return dst

            zhr = feat_ln_raw(zsum, "zhr")   # pre-LN without gain/bias
            # MLP: gelu((g.zhr)@W1 + b1') with gain folded into W1
            gel = finp.tile([D, 2, Q], F32R, name="gel")
            for f in range(2):
                h_ps = pps.tile([D, Q], F32, name="h_ps", tag="sc")
                for h in range(2):
                    nc.tensor.matmul(
                        h_ps[:, h * 512:(h + 1) * 512],
                        lhsT=wb[:, CW_W1 + f * 128:CW_W1 + (f + 1) * 128],
                        rhs=zhr[:, h * 512:(h + 1) * 512],
                        start=True, stop=True)
                nc.scalar.activation(out=gel[:, f, :], in_=h_ps, func=GELU,
                                     bias=bcol(CW_B1A + f), scale=1.0)
            o2 = pps.tile([D, Q], F32, name="o2", tag="sc")
            for h in range(2):
                for f in range(2):
                    nc.tensor.matmul(
                        o2[:, h * 512:(h + 1) * 512],
                        lhsT=wb[:, CW_W2 + f * 128:CW_W2 + (f + 1) * 128],
                        rhs=gel[:, f, h * 512:(h + 1) * 512],
                        start=(f == 0), stop=(f == 1))
            # res = pre_g*zhr + o2 + (pre_b + b2)
            res = finp.tile([D, Q], F32R, name="res")
            nc.vector.scalar_tensor_tensor(
                out=res, in0=zhr, scalar=bcol(CW_PRG), in1=o2,
                op0=MUL, op1=ADD)
            nc.vector.tensor_scalar_add(out=res, in0=res,
                                        scalar1=bcol(CW_BC2))
            fr = feat_ln_raw(res, "fr")
            final = finp.tile([D, Q], F32, name="final")
            nc.vector.tensor_scalar(out=final, in0=fr,
                                    scalar1=bcol(CW_POG),
                                    scalar2=bcol(CW_POB),
                                    op0=MUL, op1=ADD)
            nc.sync.dma_start(out=out, in_=final)

    if split:
        _split_sync_waits(nc)
    return nc


# ---------------------------------------------------------------------------
def _prep_core_inputs(b, m, q, k, v, skip, q_ln_g, q_ln_b, Wq, bq, k_ln_g,
                      k_ln_b, Wk, bk, v_ln_g, v_ln_b, Wv, bv, Wp, bp,
                      pre_g, pre_b, W1, b1, W2, b2, post_g, post_b):
    f32 = np.float32
    sl = slice(m * DH, (m + 1) * DH)

    def fold(Wm, g):
        wg = (g[:, None] * Wm)
        return (wg - wg.sum(0, keepdims=True) / 128.0).astype(f32)

    wq_ext = SCALE * fold(Wq[:, sl], q_ln_g)
    wk_ext = fold(Wk[:, sl], k_ln_g)
    wv_ext = fold(Wv[:, sl], v_ln_g)
    wbq = (SCALE * (Wq[:, sl].T @ q_ln_b + bq[sl])).astype(f32)
    u = (wk_ext @ wbq).astype(f32)
    wbv = (Wv[:, sl].T @ v_ln_b + bv[sl]).astype(f32)

    wbig = np.zeros((D, W_TOT), f32)
    wbig[:, CW_Q:CW_Q + 32] = wq_ext
    wbig[:, CW_Q + 32] = 1.0 / 128.0
    wbig[:, CW_K:CW_K + 32] = wk_ext
    wbig[:, CW_V:CW_V + 32] = wv_ext
    wbig[:, CW_V + 33] = 1.0 / 128.0
    wbig[:, CW_KM] = 1.0 / 128.0
    wbig[:, CW_KM + 1] = u
    wbig[:, CW_ONE] = 1.0
    wbig[:, CW_W1:CW_W1 + 256] = pre_g[:, None] * W1
    wbig[:, CW_W2:CW_W2 + 128] = W2[0:128, :]
    wbig[:, CW_W2 + 128:CW_W2 + 256] = W2[128:256, :]
    wbig[:, CW_BP] = bp.astype(f32)
    wbig[:, CW_BC2] = (pre_b + b2).astype(f32)
    wbig[:, CW_POG] = post_g.astype(f32)
    wbig[:, CW_POB] = post_b.astype(f32)
    wbig[:, CW_PRG] = pre_g.astype(f32)
    b1p = (pre_b @ W1 + b1).astype(f32)
    wbig[:, CW_B1A] = b1p[0:128]
    wbig[:, CW_B1B] = b1p[128:256]
    wbig[0:32, CW_WBV] = wbv

    return {
        "xq": np.ascontiguousarray(q[b].reshape(NCAM, D, Q), f32),
        "xk": np.ascontiguousarray(k[b].reshape(NCAM, D, KC), f32),
        "xv": np.ascontiguousarray(v[b].reshape(NCAM, D, KC), f32),
        "wbig": wbig,
        "wp": np.ascontiguousarray(Wp[sl, :], f32),
        "skipb": np.ascontiguousarray(skip[b].reshape(D, Q), f32),
        "eye": np.eye(D, dtype=f32),
    }


def kernel(**inputs):
    if "nc" not in _cached:
        _cached["nc"] = _build_program()
    nc = _cached["nc"]
    args = {kk: np.asarray(vv) for kk, vv in inputs.items()}
    in_maps = [_prep_core_inputs(c // 4, c % 4, **args) for c in range(N_CORES)]
    res = run_bass_kernel_spmd(nc, in_maps, core_ids=list(range(N_CORES)))
    out = np.stack([res.results[0]["out"], res.results[4]["out"]])
    return out.reshape(B, D, 32, 32)


# revision 14
# speedup vs baseline: 1.6640x; 1.1571x over previous
"""BEV cross-attention kernel for Trainium2, 8-core SPMD.

Shard: core c handles (batch b=c//4, head m=c%4). Full attention for one
(b, head): per-camera QK^T (Q=1024, K=6*1680), softmax over 10080 keys,
P@V, partial output projection; AllReduce over the 4 cores of each batch
merges heads; final skip+LN+MLP+LN computed redundantly per group.

v2 layout strategy (vs v1): all LayerNorm statistics are produced
token-major directly by tiny PE matmuls (lhsT=x chunk, rhs=ones col) --
no gpsimd cross-partition reduces and no DRAM reshape bounces. Q is
projected token-major, scaled by rstd_q per-partition, and transposed
back with PE is_transpose ops. The Q-side projection bias rides the exp
bias via u = Wk_ext @ wbq (logit bias per key), combined with ln(rstd_v)
(V LayerNorm folded through exp). Softmax denominator rides the PV
matmul as an all-ones column of V. Per-camera stages are pipelined
(load n+1 / project n / attend n-1) and the QK->exp->PV chain is
software-pipelined so the in-order PE queue never head-of-line blocks
on the Activation engine. All constants arrive in one packed DMA.
"""
import numpy as np

import concourse.bass as bass
import concourse.bass_isa as bass_isa
import concourse.mybir as mybir
import concourse.tile as tile
from concourse.bass_utils import run_bass_kernel_spmd

F32 = mybir.dt.float32
F32R = mybir.dt.float32r
BF16 = mybir.dt.bfloat16

HEADS, DH, D = 4, 32, 128
B, NCAM = 2, 6
Q = 32 * 32            # 1024 BEV queries
NQCH = Q // 128        # 8 q-chunks
KC = 28 * 60           # 1680 keys per camera
NKCH = (KC + 127) // 128   # 14 k-chunks per camera (last has 16 rows)
KFULL = KC // 128          # 13 full chunks
KTAIL = KC - KFULL * 128   # 16
N_CORES = 8
EPS = 1e-5
SCALE = DH ** -0.5

# packed weight tensor column map
CW_Q = 0           # 33 cols: s*fold(Wq g) | 1/128
CW_K = 33          # 32 cols: fold(Wk g)
CW_V = 65          # 34 cols: fold(Wv g) | 0 | 1/128
CW_KM = 99         # 2 cols: 1/128 | u  (k-mean, wbq.kh bias)
CW_ONE = 101       # 1 col: ones
CW_W1 = 104        # 256 cols: pre_g-folded W1
CW_W2 = 360        # 256 cols: W2 (two 128-row chunks)
CW_BP = 616        # f32 cols (bitcast): bp
CW_BC2 = 617       # pre_b + b2
CW_POG = 618       # post_g
CW_POB = 619       # post_b
CW_PRG = 620       # pre_g
CW_B1A = 621       # b1' half 0
CW_B1B = 622       # b1' half 1
CW_WBV = 623       # wbv (rows 0:32)
W_TOT = 624

_cached = {}


# ---------------------------------------------------------------------------
# walrus compat: this container's walrus rejects instructions carrying more
# than one semaphore wait; move excess waits onto same-engine NoOps.
_COMPUTE_ENGINES = None
_nopctr = [0]


def _split_sync_waits(nc, limit=1):
    global _COMPUTE_ENGINES
    if _COMPUTE_ENGINES is None:
        _COMPUTE_ENGINES = {
            mybir.EngineType.PE, mybir.EngineType.Activation,
            mybir.EngineType.Pool, mybir.EngineType.DVE, mybir.EngineType.SP,
        }
    for f in nc.m.functions:
        for bb in f.blocks:
            out, changed = [], False
            for inst in bb.instructions:
                si = inst.sync_info
                if (si is not None and len(si.on_wait) > limit
                        and inst.engine in _COMPUTE_ENGINES):
                    waits = list(si.on_wait)
                    n_extra = len(waits) - limit
                    for i in range(0, n_extra, limit):
                        nop = mybir.InstNoOp(name=f"wait-split-{_nopctr[0]}")
                        _nopctr[0] += 1
                        nop.engine = inst.engine
                        nop.sync_info = mybir.SyncInfo(
                            on_wait=waits[i:min(i + limit, n_extra)], on_update=[])
                        out.append(nop)
                    si.on_wait = waits[n_extra:]
                    changed = True
                out.append(inst)
            if changed:
                bb.instructions = out
    return nc


# ---------------------------------------------------------------------------
def _build_program(split=True, collective=True, n_dev=N_CORES):
    nc = bass.Bass("TRN2", target_bir_lowering=False, debug=False,
                   num_devices=n_dev)

    def din(name, shape, dt=F32R):
        return nc.dram_tensor(name, shape, dt, kind="ExternalInput").ap()

    xq = din("xq", [NCAM, D, Q])
    xk = din("xk", [NCAM, D, KC])
    xv = din("xv", [NCAM, D, KC])
    wbig = din("wbig", [D, W_TOT])
    wp = din("wp", [32, D])              # Wp head slice (lhsT)
    skipb = din("skipb", [D, 2, D], F32)   # quarter skip, token-major, +bp'
    eye = din("eye", [D, D])
    grow = din("grow", [1, D])           # post_g as a row

    QQ = Q // 4                          # per-core query quarter
    out = nc.dram_tensor("out", [D, QQ], F32, kind="ExternalOutput").ap()

    EXP = mybir.ActivationFunctionType.Exp
    LN_ = mybir.ActivationFunctionType.Ln
    SQRT = mybir.ActivationFunctionType.Sqrt
    GELU = mybir.ActivationFunctionType.Gelu
    ADD = mybir.AluOpType.add
    SUB = mybir.AluOpType.subtract
    MUL = mybir.AluOpType.mult

    with tile.TileContext(nc) as tc:
        with tc.tile_pool(name="consts", bufs=1) as consts, \
             tc.tile_pool(name="loads", bufs=2) as loads, \
             tc.tile_pool(name="sq", bufs=2) as sqp, \
             tc.tile_pool(name="sml", bufs=2) as sml, \
             tc.tile_pool(name="keep", bufs=1) as keep, \
             tc.tile_pool(name="qts", bufs=2) as qts, \
             tc.tile_pool(name="ee", bufs=3) as eep, \
             tc.tile_pool(name="fin", bufs=1) as finp, \
             tc.tile_pool(name="dramp", bufs=2, space="DRAM") as dramp, \
             tc.tile_pool(name="pps", bufs=2, space="PSUM") as pps, \
             tc.tile_pool(name="pacc", bufs=1, space="PSUM") as pacc, \
             tc.tile_pool(name="pvp", bufs=1, space="PSUM") as pvp, \
             tc.tile_pool(name="pqp", bufs=1, space="PSUM") as pqp:

            # ---- constants ----
            wb = consts.tile([D, W_TOT], F32R, name="wb")
            eye_t = consts.tile([D, D], F32R, name="eye_t")
            wp_t = consts.tile([32, D], F32R, name="wp_t")
            skip_t = consts.tile([D, 2, D], F32, name="skip_t")
            onesbc = consts.tile([1, D], F32R, name="onesbc")
            nc.vector.memset(onesbc, 1.0)
            growT = consts.tile([1, D], F32R, name="growT")

            def bcol(c, p=D):
                return wb[0:p, c:c + 1].bitcast(F32)

            # ---- persistent activations ----
            qhT = keep.tile([32, NCAM, Q], BF16, name="qhT")
            khT = keep.tile([32, NCAM, KC], BF16, name="khT")
            vhE = keep.tile([D, NCAM, NKCH, 33], BF16, name="vhE")
            rstdk = keep.tile([D, NCAM, NKCH], F32, name="rstdk")
            biask = keep.tile([D, NCAM, NKCH], F32, name="biask")

            # qp psum layout: [0:264) = 8 q-chunks x 33 (proj+mean),
            # [264:292) k mean/wbqk pairs, [292:306) k sumsq,
            # [306:320) v sumsq, [320:328) q sumsq
            QP_KM = 264
            QP_KS = 292
            QP_VS = 306
            QP_QS = 320

            loaded = {}

            def stage_load(n):
                xq_t = loads.tile([D, Q], F32R, name="xq_t", tag="xq_t")
                nc.sync.dma_start(out=xq_t, in_=xq[n])
                xk_t = loads.tile([D, KC], F32R, name="xk_t", tag="xk_t")
                nc.sync.dma_start(out=xk_t, in_=xk[n])
                xv_t = loads.tile([D, KC], F32R, name="xv_t", tag="xv_t")
                nc.sync.dma_start(out=xv_t, in_=xv[n])
                # squares: q+k on gpsimd, v on DVE (engine balance)
                x2q = sqp.tile([D, Q], F32R, name="x2q", tag="x2q")
                nc.gpsimd.scalar_tensor_tensor(
                    out=x2q, in0=xq_t, scalar=1.0, in1=xq_t, op0=MUL, op1=MUL)
                x2k = sqp.tile([D, KC], F32R, name="x2k", tag="x2k")
                nc.gpsimd.scalar_tensor_tensor(
                    out=x2k, in0=xk_t, scalar=1.0, in1=xk_t, op0=MUL, op1=MUL)
                x2v = sqp.tile([D, KC], F32R, name="x2v", tag="x2v")
                nc.vector.tensor_mul(out=x2v, in0=xv_t, in1=xv_t)
                loaded[n] = (xq_t, xk_t, xv_t, x2q, x2k, x2v)

            def stage_proj(n):
                xq_t, xk_t, xv_t, x2q, x2k, x2v = loaded.pop(n)
                qp = pqp.tile([D, 328], F32, name="qp", tag="qp")
                # Q proj token-major + q mean (col 32 of each 33-group)
                for c in range(NQCH):
                    nc.tensor.matmul(
                        qp[:, c * 33:(c + 1) * 33],
                        lhsT=xq_t[:, c * 128:(c + 1) * 128],
                        rhs=wb[:, CW_Q:CW_Q + 33], start=True, stop=True)
                    nc.tensor.matmul(
                        qp[:, QP_QS + c:QP_QS + c + 1],
                        lhsT=x2q[:, c * 128:(c + 1) * 128],
                        rhs=wb[:, CW_ONE:CW_ONE + 1], start=True, stop=True)
                # K/V stats
                for c in range(NKCH):
                    cw = 128 if c < KFULL else KTAIL
                    sl = slice(c * 128, c * 128 + cw)
                    nc.tensor.matmul(
                        qp[0:cw, QP_KM + 2 * c:QP_KM + 2 * c + 2],
                        lhsT=xk_t[:, sl], rhs=wb[:, CW_KM:CW_KM + 2],
                        start=True, stop=True)
                    nc.tensor.matmul(
                        qp[0:cw, QP_KS + c:QP_KS + c + 1],
                        lhsT=x2k[:, sl], rhs=wb[:, CW_ONE:CW_ONE + 1],
                        start=True, stop=True)
                    nc.tensor.matmul(
                        qp[0:cw, QP_VS + c:QP_VS + c + 1],
                        lhsT=x2v[:, sl], rhs=wb[:, CW_ONE:CW_ONE + 1],
                        start=True, stop=True)
                # K projection (feature-major) in two [32, 2, 512]-shaped
                # psum slots of the sc ring
                for hh in range(2):
                    kp = pps.tile([32, 2, 512], F32, name="kp", tag="sc")
                    for h2 in range(2):
                        h = hh * 2 + h2
                        nc.tensor.matmul(
                            kp[:, h2, 0:420], lhsT=wb[:, CW_K:CW_K + 32],
                            rhs=xk_t[:, h * 420:(h + 1) * 420],
                            start=True, stop=True)
                    nc.vector.tensor_copy(
                        out=khT[:, n, hh * 840:(hh + 1) * 840].rearrange(
                            "p (h c) -> p h c", h=2),
                        in_=kp[:, :, 0:420])
                # V projection (token-major), col 32 = 0 (-> ones), 33 = mean
                vp = pvp.tile([D, NKCH, 34], F32, name="vp", tag="vp")
                for c in range(NKCH):
                    cw = 128 if c < KFULL else KTAIL
                    nc.tensor.matmul(vp[0:cw, c, :],
                                     lhsT=xv_t[:, c * 128:c * 128 + cw],
                                     rhs=wb[:, CW_V:CW_V + 34],
                                     start=True, stop=True)
                nc.vector.tensor_copy(out=vhE[:, n, :, 0:32],
                                      in_=vp[:, :, 0:32])
                nc.vector.memset(vhE[:, n, :, 32], 1.0)

                # ---- rstd computations (token-major, small) ----
                def make_var(mu_ap, msq_ap, width, nm):
                    mu2 = sml.tile([128, width], F32, name=nm + "_mu2",
                                   tag=nm + "_mu2")
                    nc.vector.tensor_mul(out=mu2, in0=mu_ap, in1=mu_ap)
                    v = sml.tile([128, width], F32, name=nm + "_v",
                                 tag=nm + "_v")
                    nc.vector.scalar_tensor_tensor(
                        out=v, in0=msq_ap, scalar=1.0 / 128.0, in1=mu2,
                        op0=MUL, op1=SUB)
                    nc.vector.tensor_scalar_add(out=v, in0=v, scalar1=EPS)
                    return v

                # K: rstd_k = sqrt(1/var)
                kvar = make_var(
                    qp[:, QP_KM:QP_KM + 28].rearrange("p (c two) -> p two c",
                                                      two=2)[:, 0, :],
                    qp[:, QP_KS:QP_KS + NKCH], NKCH, "k")
                krec = sml.tile([128, NKCH], F32, name="krec", tag="krec")
                nc.vector.reciprocal(out=krec, in_=kvar)
                nc.scalar.activation(out=rstdk[:, n, :], in_=krec, func=SQRT)
                # V: bias = wbqk - 0.5*ln(var_v) (= wbqk + ln rstd_v)
                vvar = make_var(vp[:, :, 33], qp[:, QP_VS:QP_VS + NKCH],
                                NKCH, "v")
                vln = sml.tile([128, NKCH], F32, name="vln", tag="vln")
                nc.scalar.activation(out=vln, in_=vvar, func=LN_)
                nc.vector.scalar_tensor_tensor(
                    out=biask[:, n, :], in0=vln, scalar=-0.5,
                    in1=qp[:, QP_KM:QP_KM + 28].rearrange(
                        "p (c two) -> p two c", two=2)[:, 1, :],
                    op0=MUL, op1=ADD)
                # Q: rstd_q, then scale token-major projections
                qvar = make_var(
                    qp[:, 0:264].rearrange("p (c w) -> p w c", w=33)[:, 32, :],
                    qp[:, QP_QS:QP_QS + NQCH], NQCH, "q")
                qrec = sml.tile([128, NQCH], F32, name="qrec", tag="qrec")
                nc.vector.reciprocal(out=qrec, in_=qvar)
                rstdq = sml.tile([128, NQCH], F32, name="rstdq", tag="rstdq")
                nc.scalar.activation(out=rstdq, in_=qrec, func=SQRT)
                qpTs = qts.tile([D, NQCH, 32], F32R, name="qpTs", tag="qpTs")
                for c in range(NQCH):
                    nc.vector.tensor_scalar_mul(
                        out=qpTs[:, c, :], in0=qp[:, c * 33:c * 33 + 32],
                        scalar1=rstdq[:, c:c + 1])
                # transpose back to feature-major [32, Q]
                qT = pps.tile([32, Q], F32R, name="qT", tag="sc")
                for c in range(NQCH):
                    nc.tensor.transpose(out=qT[:, c * 128:(c + 1) * 128],
                                        in_=qpTs[:, c, :], identity=eye_t)
                nc.vector.tensor_copy(out=qhT[:, n, :], in_=qT)

            # ---- attention chunk list, software pipelined ----
            avt = pacc.tile([33, Q], F32, name="avt")
            chunks = [(n, c) for n in range(NCAM) for c in range(NKCH)]
            pend = []  # (et, n, c, first)

            def emit_qk(i):
                n, c = chunks[i]
                cw = 128 if c < KFULL else KTAIL
                sc = pps.tile([128, Q], F32, name="sc", tag="sc")
                for h in range(2):
                    nc.tensor.matmul(
                        sc[0:cw, h * 512:(h + 1) * 512],
                        lhsT=khT[:, n, c * 128:c * 128 + cw],
                        rhs=qhT[:, n, h * 512:(h + 1) * 512],
                        start=True, stop=True)
                et = eep.tile([128, Q], BF16, name="et", tag="et")
                nc.scalar.activation(out=et[0:cw, :], in_=sc[0:cw, :],
                                     func=EXP,
                                     bias=biask[0:cw, n, c:c + 1],
                                     scale=rstdk[0:cw, n, c:c + 1])
                pend.append((et, n, c, i == 0))

            def emit_pv(last=False):
                et, n, c, first = pend.pop(0)
                cw = 128 if c < KFULL else KTAIL
                for h in range(2):
                    nc.tensor.matmul(
                        avt[:, h * 512:(h + 1) * 512],
                        lhsT=vhE[0:cw, n, c, 0:33],
                        rhs=et[0:cw, h * 512:(h + 1) * 512],
                        start=first, stop=last)

            # ---- stage-pipelined main loop ----
            stage_load(0)
            nc.sync.dma_start(out=wb, in_=wbig)
            nc.sync.dma_start(out=eye_t, in_=eye)
            stage_proj(0)
            stage_load(1)
            nc.sync.dma_start(out=wp_t, in_=wp)
            nc.sync.dma_start(out=skip_t, in_=skipb)
            nc.sync.dma_start(out=growT, in_=grow)
            ci = 0
            for n in range(NCAM):
                for c in range(NKCH):
                    emit_qk(ci)
                    ci += 1
                    if len(pend) > 1:
                        emit_pv(last=False)
                    if c == 3 and n + 1 < NCAM:
                        stage_proj(n + 1)
                    if c == 8 and n + 2 < NCAM:
                        stage_load(n + 2)
            while pend:
                emit_pv(last=len(pend) == 1)

            # ---- normalize + output projection (transposed out) ----
            avt_sb = finp.tile([33, Q], F32, name="avt_sb")
            nc.vector.tensor_copy(out=avt_sb, in_=avt)
            rden = finp.tile([1, Q], F32, name="rden", tag="lnrs")
            nc.vector.reciprocal(out=rden, in_=avt_sb[32:33, :])
            rd_bc = pps.tile([32, Q], F32, name="rd_bc", tag="sc")
            for h in range(2):
                nc.tensor.matmul(rd_bc[:, h * 512:(h + 1) * 512],
                                 lhsT=onesbc[:, 0:32],
                                 rhs=rden[:, h * 512:(h + 1) * 512].bitcast(F32R),
                                 start=True, stop=True)
            anorm = finp.tile([32, Q], F32R, name="anorm")
            nc.vector.tensor_mul(out=anorm, in0=avt_sb[0:32, :], in1=rd_bc)
            # zpT[tok, d] partial output projection, token-major
            zpT = pps.tile([D, NQCH, D], F32, name="zpT", tag="sc")
            for c in range(NQCH):
                nc.tensor.matmul(zpT[:, c, :],
                                 lhsT=anorm[:, c * 128:(c + 1) * 128],
                                 rhs=wp_t, start=True, stop=True)
            zpT_sb = finp.tile([D, NQCH, D], F32, name="zpT_sb")
            nc.vector.tensor_copy(out=zpT_sb, in_=zpT)
            zpart = dramp.tile([Q, D], F32, name="zpart")
            zred = dramp.tile([QQ, D], F32, name="zred")
            nc.sync.dma_start(
                out=zpart.rearrange("(c p) d -> p c d", p=128), in_=zpT_sb)
            if collective:
                nc.gpsimd.collective_compute(
                    "ReduceScatter", mybir.AluOpType.add,
                    replica_groups=[[0, 1, 2, 3], [4, 5, 6, 7]],
                    ins=[zpart.opt()], outs=[zred.opt()],
                )
            else:
                nc.sync.dma_start(out=zred, in_=zpart[0:QQ, :])

            # ---- final quarter: skip + pre-LN + MLP + post-LN ----
            zt = finp.tile([D, 2, D], F32, name="zt")
            nc.sync.dma_start(
                out=zt, in_=zred.rearrange("(h p) d -> p h d", p=128))
            zsum = finp.tile([D, 2, D], F32R, name="zsum")
            nc.vector.tensor_add(out=zsum, in0=zt, in1=skip_t)

            def tm_stats(src_tm, nm):
                """token-major LN stats: src [128, 2, 128] -> mu, rs [128, 2]."""
                sq = finp.tile([D, 2, D], F32R, name=nm + "_sq", tag="fsq")
                nc.vector.tensor_mul(out=sq, in0=src_tm, in1=src_tm)
                sm = sml.tile([128, 2, 2], F32, name=nm + "_sm", tag="fsm")
                nc.vector.tensor_reduce(out=sm[:, :, 0], in_=src_tm,
                                        axis=mybir.AxisListType.X, op=ADD)
                nc.vector.tensor_reduce(out=sm[:, :, 1], in_=sq,
                                        axis=mybir.AxisListType.X, op=ADD)
                mu = sml.tile([128, 2], F32, name=nm + "_mu", tag="fmu")
                nc.vector.tensor_scalar_mul(out=mu, in0=sm[:, :, 0],
                                            scalar1=1.0 / 128.0)
                mu2 = sml.tile([128, 2], F32, name=nm + "_mu2", tag="fmu2")
                nc.vector.tensor_mul(out=mu2, in0=mu, in1=mu)
                var = sml.tile([128, 2], F32, name=nm + "_var", tag="fvar")
                nc.vector.scalar_tensor_tensor(
                    out=var, in0=sm[:, :, 1], scalar=1.0 / 128.0, in1=mu2,
                    op0=MUL, op1=SUB)
                nc.vector.tensor_scalar_add(out=var, in0=var, scalar1=EPS)
                rec = sml.tile([128, 2], F32, name=nm + "_rec", tag="frec")
                nc.vector.reciprocal(out=rec, in_=var)
                rs = sml.tile([128, 2], F32, name=nm + "_rs", tag="frs")
                nc.scalar.activation(out=rs, in_=rec, func=SQRT)
                return mu, rs

            mu1, rs1 = tm_stats(zsum, "l1")
            zhat_tm = finp.tile([D, 2, D], F32R, name="zhat_tm")
            for h in range(2):
                nc.vector.tensor_scalar(
                    out=zhat_tm[:, h, :], in0=zsum[:, h, :],
                    scalar1=mu1[:, h:h + 1], scalar2=rs1[:, h:h + 1],
                    op0=SUB, op1=MUL)
            # transpose to feature-major for the MLP
            zf_ps = pps.tile([D, 2 * D], F32R, name="zf_ps", tag="sc")
            for h in range(2):
                nc.tensor.transpose(out=zf_ps[:, h * D:(h + 1) * D],
                                    in_=zhat_tm[:, h, :], identity=eye_t)
            zhat = finp.tile([D, 2 * D], F32R, name="zhat")
            nc.vector.tensor_copy(out=zhat, in_=zf_ps)
            gel = finp.tile([D, 2, 2 * D], F32R, name="gel")
            for f in range(2):
                h_ps = pps.tile([D, 2 * D], F32, name="h_ps", tag="sc")
                nc.tensor.matmul(
                    h_ps,
                    lhsT=wb[:, CW_W1 + f * 128:CW_W1 + (f + 1) * 128],
                    rhs=zhat, start=True, stop=True)
                nc.scalar.activation(out=gel[:, f, :], in_=h_ps, func=GELU,
                                     bias=bcol(CW_B1A + f), scale=1.0)
            o2 = pps.tile([D, 2 * D], F32, name="o2", tag="sc")
            for f in range(2):
                nc.tensor.matmul(
                    o2, lhsT=wb[:, CW_W2 + f * 128:CW_W2 + (f + 1) * 128],
                    rhs=gel[:, f, :], start=(f == 0), stop=(f == 1))
            # res = pre_g*zhat + o2 + (pre_b + b2)
            res = finp.tile([D, 2 * D], F32R, name="res")
            nc.vector.scalar_tensor_tensor(
                out=res, in0=zhat, scalar=bcol(CW_PRG), in1=o2,
                op0=MUL, op1=ADD)
            nc.vector.tensor_scalar_add(out=res, in0=res,
                                        scalar1=bcol(CW_BC2))
            # post-LN: stats via per-quarter rows, apply feature-major
            s2 = finp.tile([D, 2 * D], F32R, name="s2")
            nc.vector.tensor_mul(out=s2, in0=res, in1=res)
            srow = pps.tile([33, 2 * D], F32, name="srow", tag="sc")
            nc.tensor.matmul(srow[0:1, :], lhsT=wb[:, CW_ONE:CW_ONE + 1],
                             rhs=res, start=True, stop=True)
            nc.tensor.matmul(srow[32:33, :], lhsT=wb[:, CW_ONE:CW_ONE + 1],
                             rhs=s2, start=True, stop=True)
            rows = finp.tile([4, 2 * D], F32, name="rows", tag="rows")
            mu = rows[0:1, :]
            nc.vector.tensor_scalar_mul(out=mu, in0=srow[0:1, :],
                                        scalar1=1.0 / 128.0)
            mu2 = rows[1:2, :]
            nc.vector.tensor_mul(out=mu2, in0=mu, in1=mu)
            var = rows[2:3, :]
            nc.vector.scalar_tensor_tensor(
                out=var, in0=srow[32:33, :], scalar=1.0 / 128.0, in1=mu2,
                op0=MUL, op1=SUB)
            nc.vector.tensor_scalar_add(out=var, in0=var, scalar1=EPS)
            rec = rows[3:4, :]
            nc.vector.reciprocal(out=rec, in_=var)
            rs = finp.tile([1, 2 * D], F32, name="rs_f", tag="lnrs")
            nc.scalar.activation(out=rs, in_=rec, func=SQRT)
            mu_bc = pps.tile([D, 2 * D], F32, name="mu_bc", tag="sc")
            rs_bc = pps.tile([D, 2 * D], F32, name="rs_bc", tag="sc")
            nc.tensor.matmul(mu_bc, lhsT=onesbc, rhs=mu.bitcast(F32R),
                             start=True, stop=True)
            nc.tensor.matmul(rs_bc, lhsT=growT, rhs=rs.bitcast(F32R),
                             start=True, stop=True)
            zc = finp.tile([D, 2 * D], F32, name="zc")
            nc.vector.tensor_sub(out=zc, in0=res, in1=mu_bc)
            final = finp.tile([D, 2 * D], F32, name="final")
            nc.vector.scalar_tensor_tensor(
                out=final, in0=zc, scalar=1.0, in1=rs_bc, op0=MUL, op1=MUL)
            nc.vector.tensor_scalar_add(out=final, in0=final,
                                        scalar1=bcol(CW_POB))
            nc.sync.dma_start(out=out, in_=final)

    if split:
        _split_sync_waits(nc)
    return nc


# ---------------------------------------------------------------------------
def _prep_core_inputs(b, m, q, k, v, skip, q_ln_g, q_ln_b, Wq, bq, k_ln_g,
                      k_ln_b, Wk, bk, v_ln_g, v_ln_b, Wv, bv, Wp, bp,
                      pre_g, pre_b, W1, b1, W2, b2, post_g, post_b):
    f32 = np.float32
    sl = slice(m * DH, (m + 1) * DH)

    def fold(Wm, g):
        wg = (g[:, None] * Wm)
        return (wg - wg.sum(0, keepdims=True) / 128.0).astype(f32)

    wq_ext = SCALE * fold(Wq[:, sl], q_ln_g)
    wk_ext = fold(Wk[:, sl], k_ln_g)
    wv_ext = fold(Wv[:, sl], v_ln_g)
    wbq = (SCALE * (Wq[:, sl].T @ q_ln_b + bq[sl])).astype(f32)
    u = (wk_ext @ wbq).astype(f32)

    wbig = np.zeros((D, W_TOT), f32)
    wbig[:, CW_Q:CW_Q + 32] = wq_ext
    wbig[:, CW_Q + 32] = 1.0 / 128.0
    wbig[:, CW_K:CW_K + 32] = wk_ext
    wbig[:, CW_V:CW_V + 32] = wv_ext
    wbig[:, CW_V + 33] = 1.0 / 128.0
    wbig[:, CW_KM] = 1.0 / 128.0
    wbig[:, CW_KM + 1] = u
    wbig[:, CW_ONE] = 1.0
    wbig[:, CW_W1:CW_W1 + 256] = pre_g[:, None] * W1
    wbig[:, CW_W2:CW_W2 + 128] = W2[0:128, :]
    wbig[:, CW_W2 + 128:CW_W2 + 256] = W2[128:256, :]
    wbig[:, CW_BC2] = (pre_b + b2).astype(f32)
    wbig[:, CW_POG] = post_g.astype(f32)
    wbig[:, CW_POB] = post_b.astype(f32)
    wbig[:, CW_PRG] = pre_g.astype(f32)
    b1p = (pre_b @ W1 + b1).astype(f32)
    wbig[:, CW_B1A] = b1p[0:128]
    wbig[:, CW_B1B] = b1p[128:256]

    # bias of the output projection, plus every head's folded wbv term,
    # folded into this core's quarter of the (token-major) skip tensor
    bpp = bp.copy()
    for mm in range(HEADS):
        slm = slice(mm * DH, (mm + 1) * DH)
        wbv_m = Wv[:, slm].T @ v_ln_b + bv[slm]
        bpp = bpp + Wp[slm, :].T @ wbv_m
    skip_q = skip[b].reshape(D, Q)[:, m * (Q // 4):(m + 1) * (Q // 4)]
    skip_tm = skip_q.T.reshape(2, 128, D).transpose(1, 0, 2) + bpp[None, None, :]

    return {
        "xq": np.ascontiguousarray(q[b].reshape(NCAM, D, Q), f32),
        "xk": np.ascontiguousarray(k[b].reshape(NCAM, D, KC), f32),
        "xv": np.ascontiguousarray(v[b].reshape(NCAM, D, KC), f32),
        "wbig": wbig,
        "wp": np.ascontiguousarray(Wp[sl, :], f32),
        "skipb": np.ascontiguousarray(skip_tm, f32),
        "eye": np.eye(D, dtype=f32),
        "grow": post_g.astype(f32).reshape(1, D),
    }


def kernel(**inputs):
    if "nc" not in _cached:
        _cached["nc"] = _build_program()
    nc = _cached["nc"]
    args = {kk: np.asarray(vv) for kk, vv in inputs.items()}
    in_maps = [_prep_core_inputs(c // 4, c % 4, **args) for c in range(N_CORES)]
    res = run_bass_kernel_spmd(nc, in_maps, core_ids=list(range(N_CORES)))
    out = np.empty((B, D, Q), np.float32)
    for c in range(N_CORES):
        b, m = c // 4, c % 4
        out[b, :, m * (Q // 4):(m + 1) * (Q // 4)] = res.results[c]["out"]
    return out.reshape(B, D, 32, 32)
